# revision 1
# baseline (speedup 1.0000x reference)
"""Trainium2 Bass kernel for nn_ConstraintsModule (fuzzy-logic constraint
propagation).

Algorithm notes
---------------
The reference computes, twice (apply-1 with active=full_body, apply-2 with
active=unsat_head and goal-masked bodies):

    body_rev[b,c,a] = pb[c,a] + v[b,a]*(nb-pb)      -> max over a
    body_min[b,c]   = active[b,c] * (1 - max_a body_rev)
    lb[b,n] = max_c body_min * pos_head[c,n] ; ub = 1 - max_c body_min*neg_head
    u = max(min(lb,ub), min(max(lb,ub), v))

Because bodies are sparse (~4 literals/constraint) and heads are one-hot,
the dense [B, C, NA] tensor never needs to exist:

  max_a body_rev[b,c,:] = max over the constraint's literal list of
      v+[b,a] (pos literals) / v-[b,a] (neg literals)
  where apply-1: v+ = 1-p, v- = p ; apply-2: v+ = (1-g)(1-u1), v- = g*u1.

Sharding: constraints are owned by the core that owns their head atom
(atom range of 128 per core), so the head-scatter and clamp are core-local.
The device runs ONE compiled SPMD program twice (the two applies); the host
gathers per-literal value rows between launches (pure data layout) and
re-feeds them.  All reductions / matmuls / compares / clamps run on device:

  per core & launch:
    W[s,b]   = reduce_max over padded literal slots       (DVE)
    act[s,b] = (sum_a sgT[a,b]*lhsT[a,s] == target[s])    (PE bf16 + DVE cmp)
    bm       = act * (1 - W)                              (DVE)
    lb/ub    = one-hot scatter matmuls per collision layer (PE fp32, exact)
               + max across layers                        (DVE)
    u_slice  = max(min(lb,ub), min(max(lb,ub), base))     (DVE)
"""
import numpy as np

import concourse.bass as bass
import concourse.tile as tile
from concourse import mybir
from concourse.tile import ScopedClock
from concourse.bass_utils import run_bass_kernel_spmd

B = 128
NCOL = 2048
NA = 1024
C = 512
NCORES = 8
SLOTS = 128          # constraint slots per core (padded)
NLOC = 128           # atoms per core
KC = NA // 128       # contraction chunks for the active matmul


class FixedTileContext(tile.TileContext):
    """Two workarounds for this walrus/NRT combo: (1) skip the tail
    clear_and_free_semaphores — its InstSemClear makes NRT reject the NEFF at
    load, and NRT resets semaphores per execution anyway; (2) multi-wait
    instructions are split afterwards by split_multi_waits()."""

    def _drain_and_barrier(self, tick_clock, wait_clock):
        drain_inst = self.nc.sync.drain()
        wait_clock.add_sem_waits(
            drain_inst.ins, ScopedClock({None: tick_clock.global_clock})
        )
        self.nc.all_engine_barrier()
        assert self.sems is not None
        popped = self.nc._tile_sem_poison_stack.pop()
        assert popped is self._sem_poison
        self.nc.all_engine_barrier()


def split_multi_waits(nc: bass.Bass) -> int:
    """walrus here accepts only ONE sync wait per instruction; Tile's
    add_semaphores attaches several.  Hoist all but one wait onto fresh
    same-engine nops placed immediately before the instruction (engine
    program order is preserved, so blocking semantics are identical)."""
    n_split = 0
    for f in nc.m.functions:
        for b in f.blocks:
            new = []
            for ins in b.instructions:
                si = ins.sync_info
                waits = list(si.on_wait) if si and si.on_wait else []
                if len(waits) > 1:
                    for w in waits[:-1]:
                        nop = mybir.InstNoOp(
                            name=f"waitsplit-{n_split}", ins=[], outs=[])
                        n_split += 1
                        nop.engine = ins.engine
                        nop.sync_info = mybir.SyncInfo(on_wait=[w], on_update=[])
                        new.append(nop)
                    ins.sync_info = mybir.SyncInfo(
                        on_wait=[waits[-1]],
                        on_update=list(si.on_update) if si.on_update else [])
                new.append(ins)
            b.instructions = new
    return n_split


_PROGRAM_CACHE = {}
SPLIT_WAITS = True  # set False when running under CoreSim (sim chokes on the
                    # synthesized nops, and doesn't need the split anyway)


def _build_program(kpad: tuple, lpos: int, lneg: int) -> bass.Bass:
    """One SPMD apply phase.  Inputs are per-core; same program serves both
    applies (the lhsT / target / vperm / base inputs differ per launch).
    kpad = (k_hi, k_lo): slots are sorted by literal count, slots 0:64 use
    k_hi literal rows, slots 64:128 use k_lo."""
    key = (kpad, lpos, lneg)
    if key in _PROGRAM_CACHE:
        return _PROGRAM_CACHE[key]

    f32, bf16 = mybir.dt.float32, mybir.dt.bfloat16
    nc = bass.Bass(num_devices=NCORES)
    k_hi, k_lo = kpad
    vph_d = nc.declare_dram_parameter("vph", [64, k_hi * B], f32, isOutput=False)
    vpl_d = nc.declare_dram_parameter("vpl", [64, k_lo * B], f32, isOutput=False)
    # (two-group layout kept: slots sorted by literal count)
    sg_d = nc.declare_dram_parameter("sg", [128, KC * B], bf16, isOutput=False)
    lhsT_d = nc.declare_dram_parameter("lhsT", [128, KC * SLOTS], bf16, isOutput=False)
    targ_d = nc.declare_dram_parameter("targ", [SLOTS, 1], f32, isOutput=False)
    scat_d = nc.declare_dram_parameter(
        "scat", [SLOTS, (lpos + lneg) * NLOC], bf16, isOutput=False)
    base_d = nc.declare_dram_parameter("base", [NLOC, B], f32, isOutput=False)
    u_d = nc.declare_dram_parameter("u", [NLOC, B], f32, isOutput=True)

    with FixedTileContext(nc) as tc:
        with (
            tc.tile_pool(name="sbuf", bufs=1) as pool,
            tc.tile_pool(name="psum", bufs=1, space="PSUM") as psum,
        ):
            # Spread input loads across the two HWDGE rings (sync, scalar) and
            # SWDGE (gpsimd) so they don't serialize on one queue; PE-critical
            # tensors (lhsT, sg) go first on their ring.
            sg = pool.tile([128, KC, B], bf16)
            nc.sync.dma_start(sg[:], sg_d[:].rearrange("p (k b) -> p k b", k=KC))
            lh = pool.tile([128, KC, SLOTS], bf16)
            nc.sync.dma_start(lh[:], lhsT_d[:].rearrange("p (k s) -> p k s", k=KC))
            tg = pool.tile([SLOTS, 1], f32)
            nc.sync.dma_start(tg[:], targ_d[:])
            # vperm: slots sorted by literal count; the bottom 64 slots
            # need far fewer literal rows -> much smaller load + reduce
            vph = pool.tile([64, k_hi, B], f32)
            nc.scalar.dma_start(vph[:], vph_d[:].rearrange("s (k b) -> s k b", k=k_hi))
            vpl = pool.tile([64, k_lo, B], f32)
            nc.scalar.dma_start(vpl[:], vpl_d[:].rearrange("s (k b) -> s k b", k=k_lo))
            sc_b = pool.tile([SLOTS, lpos + lneg, NLOC], bf16)
            nc.sync.dma_start(
                sc_b[:], scat_d[:].rearrange("s (l n) -> s l n", l=lpos + lneg))
            sc = pool.tile([SLOTS, lpos + lneg, NLOC], f32)
            nc.scalar.copy(sc[:], sc_b[:])  # exact 0/1 upcast, off critical path
            bs = pool.tile([NLOC, B], f32)
            nc.scalar.dma_start(bs[:], base_d[:])

            # W[s,b] = max over literal slots (padding rows are 0.0)
            w = pool.tile([SLOTS, B], f32)
            nc.vector.tensor_reduce(
                out=w[:64, :], in_=vph[:].rearrange("s k b -> s b k"),
                axis=mybir.AxisListType.X, op=mybir.AluOpType.max)
            nc.vector.tensor_reduce(
                out=w[64:, :], in_=vpl[:].rearrange("s k b -> s b k"),
                axis=mybir.AxisListType.X, op=mybir.AluOpType.max)

            # act[s,b] = (sum_a lhsT[a,s]*sg[a,b] == targ[s])
            ps_act = psum.tile([SLOTS, B], f32)
            for k in range(KC):
                nc.tensor.matmul(
                    ps_act[:], lh[:, k, :], sg[:, k, :],
                    start=(k == 0), stop=(k == KC - 1))
            act = pool.tile([SLOTS, B], f32)
            nc.vector.tensor_scalar(
                act[:], ps_act[:], tg[:], None, mybir.AluOpType.is_equal)

            # bm = act * (1 - W)
            omw = pool.tile([SLOTS, B], f32)
            nc.vector.tensor_scalar(
                omw[:], w[:], -1.0, 1.0, mybir.AluOpType.mult, mybir.AluOpType.add)
            bm = pool.tile([SLOTS, B], f32)
            nc.vector.tensor_tensor(bm[:], act[:], omw[:], mybir.AluOpType.mult)

            # head scatter: lb = max over pos layers, nmax = max over neg layers
            def scatter_max(l0, nlayers, name):
                tiles = []
                for l in range(nlayers):
                    pt = psum.tile([NLOC, B], f32, tag=f"{name}{l}")
                    nc.tensor.matmul(pt[:], sc[:, l0 + l, :], bm[:],
                                     start=True, stop=True)
                    tiles.append(pt)
                # tensor_tensor may read at most one PSUM operand; do the
                # PSUM->SBUF copy on the otherwise-idle Scalar engine
                acc = pool.tile([NLOC, B], f32, tag=f"{name}acc")
                nc.scalar.copy(acc[:], tiles[0][:])
                for l in range(1, nlayers):
                    nxt = pool.tile([NLOC, B], f32, tag=f"{name}acc{l}")
                    nc.vector.tensor_tensor(
                        nxt[:], acc[:], tiles[l][:], mybir.AluOpType.max)
                    acc = nxt
                return acc

            lb = scatter_max(0, lpos, "sp")
            nmax = scatter_max(lpos, lneg, "sn")
            ub = pool.tile([NLOC, B], f32)
            nc.vector.tensor_scalar(
                ub[:], nmax[:], -1.0, 1.0, mybir.AluOpType.mult, mybir.AluOpType.add)

            lo = pool.tile([NLOC, B], f32)
            nc.vector.tensor_tensor(lo[:], lb[:], ub[:], mybir.AluOpType.min)
            hi = pool.tile([NLOC, B], f32)
            nc.vector.tensor_tensor(hi[:], lb[:], ub[:], mybir.AluOpType.max)
            mid = pool.tile([NLOC, B], f32)
            nc.vector.tensor_tensor(mid[:], hi[:], bs[:], mybir.AluOpType.min)
            u = pool.tile([NLOC, B], f32)
            nc.vector.tensor_tensor(u[:], lo[:], mid[:], mybir.AluOpType.max)
            nc.sync.dma_start(u_d[:], u[:])

    if SPLIT_WAITS:
        split_multi_waits(nc)
    _PROGRAM_CACHE[key] = nc
    return nc


class _Prep:
    """Host-side, input-value-independent-of-u preprocessing (everything that
    doesn't depend on intermediate u1)."""

    def __init__(self, preds, goal, atoms, pos_body, neg_body, pos_head, neg_head):
        f32 = np.float32
        self.atoms = np.asarray(atoms)
        self.p = preds[:, self.atoms].astype(f32)            # [B, NA]
        self.g = goal[:, self.atoms].astype(f32)
        self.pT = np.ascontiguousarray(self.p.T)             # [NA, B]
        self.gT = np.ascontiguousarray(self.g.T)

        import ml_dtypes
        self.bf16 = ml_dtypes.bfloat16
        sgT = (2.0 * self.g - 1.0).T                         # [NA, B]
        self.sg_dev = np.ascontiguousarray(
            sgT.reshape(KC, 128, B).transpose(1, 0, 2).reshape(128, KC * B)
        ).astype(self.bf16)

        hsum = pos_head + neg_head
        assert np.all(hsum.sum(axis=1) == 1.0), "heads must be one-hot"
        self.h = np.argmax(hsum, axis=1)                     # [C]
        self.head_is_pos = pos_head[np.arange(C), self.h] == 1.0
        owner = self.h // NLOC

        symm_body = (pos_body - neg_body).astype(f32)        # [C, NA]
        symm_head = (pos_head - neg_head).astype(f32)
        lit_count = (pos_body + neg_body).sum(axis=1).astype(f32)

        # literal row lists (row space: a -> v+ region, NA+a -> v- region)
        pos_lists = [np.nonzero(pos_body[c])[0] for c in range(C)]
        neg_lists = [np.nonzero(neg_body[c])[0] for c in range(C)]
        ncnt = np.array([len(pos_lists[c]) + len(neg_lists[c]) for c in range(C)])

        self.cons = []        # per core: constraint ids in slot order
        self.rows = []        # per core: [SLOTS, kpad] int row ids (-1 = pad)
        self.lhsTb = []       # per core: [128, KC*SLOTS] bf16 (symm_body)
        self.lhsTh = []       # per core: [128, KC*SLOTS] bf16 (symm_head)
        self.targ1 = []
        self.targ2 = []
        lpos_need, lneg_need = 1, 1
        layer_asn = []        # per core: (slot, is_pos, layer, nloc) list
        k_hi = k_lo = 1
        for i in range(NCORES):
            ci = np.nonzero(owner == i)[0]
            assert len(ci) <= SLOTS, f"core {i} has {len(ci)} constraints"
            # sort slots by literal count (desc): slots 64:128 then need far
            # fewer padded literal rows than slots 0:64
            ci = ci[np.argsort(-ncnt[ci], kind="stable")]
            self.cons.append(ci)
            cnts = ncnt[ci]
            k_hi = max(k_hi, int(cnts[:64].max(initial=0)))
            k_lo = max(k_lo, int(cnts[64:].max(initial=0)))
        self.kpad = (k_hi, k_lo)
        for i in range(NCORES):
            ci = self.cons[i]
            rows = -np.ones((SLOTS, k_hi), dtype=np.int64)
            for s, c in enumerate(ci):
                rr = np.concatenate([pos_lists[c], NA + neg_lists[c]])
                rows[s, : len(rr)] = rr
            self.rows.append(rows)

            def pack_lhsT(m):
                sl = np.zeros((NA, SLOTS), dtype=f32)
                sl[:, : len(ci)] = m[ci].T
                return np.ascontiguousarray(
                    sl.reshape(KC, 128, SLOTS).transpose(1, 0, 2)
                    .reshape(128, KC * SLOTS)).astype(self.bf16)

            self.lhsTb.append(pack_lhsT(symm_body))
            self.lhsTh.append(pack_lhsT(symm_head))
            t1 = np.full((SLOTS, 1), 1e9, dtype=f32)
            t1[: len(ci), 0] = lit_count[ci]
            self.targ1.append(t1)
            t2 = np.full((SLOTS, 1), 1e9, dtype=f32)
            t2[: len(ci), 0] = -1.0
            self.targ2.append(t2)

            # collision layers for the head scatter
            counts = {}
            asn = []
            for s, c in enumerate(ci):
                key = (self.h[c] % NLOC, bool(self.head_is_pos[c]))
                l = counts.get(key, 0)
                counts[key] = l + 1
                asn.append((s, key[1], l, key[0]))
                if key[1]:
                    lpos_need = max(lpos_need, l + 1)
                else:
                    lneg_need = max(lneg_need, l + 1)
            layer_asn.append(asn)

        self.lpos, self.lneg = lpos_need, lneg_need
        self.scat = []
        for i in range(NCORES):
            sc = np.zeros((SLOTS, self.lpos + self.lneg, NLOC), dtype=f32)
            for s, is_pos, l, n in layer_asn[i]:
                li = l if is_pos else self.lpos + l
                sc[s, li, n] = 1.0
            self.scat.append(np.ascontiguousarray(
                sc.reshape(SLOTS, -1)).astype(self.bf16))

    def vperm_maps(self, vcat: np.ndarray):
        """vcat: [2*NA, B] value table -> per-core (vph, vpl) f32 arrays."""
        k_hi, k_lo = self.kpad
        out = []
        vext = np.concatenate([vcat, np.zeros((1, B), np.float32)], axis=0)
        for i in range(NCORES):
            rows = self.rows[i]  # -1 pads -> last (zero) row
            g = vext[rows]       # [SLOTS, k_hi, B]
            vph = np.ascontiguousarray(
                g[:64].reshape(64, k_hi * B)).astype(np.float32)
            vpl = np.ascontiguousarray(
                g[64:, :k_lo].reshape(64, k_lo * B)).astype(np.float32)
            out.append((vph, vpl))
        return out


def kernel(preds, goal, atoms, pos_body, neg_body, pos_head, neg_head):
    preds = np.asarray(preds)
    prep = _Prep(np.asarray(preds, np.float32), np.asarray(goal, np.float32),
                 atoms, np.asarray(pos_body, np.float32),
                 np.asarray(neg_body, np.float32),
                 np.asarray(pos_head, np.float32),
                 np.asarray(neg_head, np.float32))
    nc = _build_program(prep.kpad, prep.lpos, prep.lneg)
    core_ids = list(range(NCORES))

    def launch(vcat, lhsT_list, targ_list, baseT):
        vperms = prep.vperm_maps(vcat)
        in_maps = []
        for i in range(NCORES):
            vph_i, vpl_i = vperms[i]
            in_maps.append({
                "vph": vph_i,
                "vpl": vpl_i,
                "sg": prep.sg_dev,
                "lhsT": lhsT_list[i],
                "targ": targ_list[i],
                "scat": prep.scat[i],
                "base": np.ascontiguousarray(
                    baseT[i * NLOC:(i + 1) * NLOC]).astype(np.float32),
            })
        res = run_bass_kernel_spmd(nc, in_maps, core_ids)
        return np.concatenate(
            [res.results[i]["u"] for i in range(NCORES)], axis=0)  # [NA, B]

    # apply 1: v+ = 1-p, v- = p, active vs lit_count, base = p
    vcat1 = np.concatenate([1.0 - prep.pT, prep.pT], axis=0)
    u1T = launch(vcat1, prep.lhsTb, prep.targ1, prep.pT)

    # apply 2: v+ = (1-g)(1-u1), v- = g*u1, active vs -1 (head), base = u1
    vcat2 = np.concatenate(
        [(1.0 - prep.gT) * (1.0 - u1T), prep.gT * u1T], axis=0
    ).astype(np.float32)
    u2T = launch(vcat2, prep.lhsTh, prep.targ2, u1T)

    out = np.array(preds, dtype=preds.dtype, copy=True)
    out[:, prep.atoms] = u2T.T.astype(preds.dtype)
    return out



# revision 28
# speedup vs baseline: 1.4351x; 1.4351x over previous
"""Trainium2 Bass kernel for nn_ConstraintsModule (fuzzy-logic constraint
propagation).

Algorithm notes
---------------
The reference computes, twice (apply-1 with active=full_body, apply-2 with
active=unsat_head and goal-masked bodies):

    body_rev[b,c,a] = pb[c,a] + v[b,a]*(nb-pb)      -> max over a
    body_min[b,c]   = active[b,c] * (1 - max_a body_rev)
    lb[n] = max_c body_min * pos_head[c,n] ; ub = 1 - max_c body_min*neg_head
    u = max(min(lb,ub), min(max(lb,ub), v))

Bodies are sparse (~4 literals/constraint) and heads one-hot, so per
constraint we only gather its literal-value rows and min-reduce their
complements:  bm = 1 - max_a(v) = min_a(1 - v).

Key tricks:
1. The `active` gate folds into the gathered VALUES: a literal row whose
   goal-condition fails gets complement value -1, making bm <= 0, and
   relu() at the scatter stage reproduces active=0 exactly.  For apply-2
   (active = head literal unsatisfied by goal) one extra "head
   activation" row is appended per constraint.  This removes the
   goal@body equality matmul and its big operand loads completely.
2. Precision: apply-2 consumes BOTH u1 and 1-u1, so launch 1 runs f32
   end-to-end (table rows are exact f32 complements; the f32 bm crosses
   the PE as a split bf16 pair hi+lo accumulated in PSUM, keeping rel
   error ~1.6e-5).  Launch 2 only needs u2 itself to be accurate, so it
   runs bf16 with head-sign-specific value spaces: pos-head slots store
   complements v'=1-v (bm selections), neg-head slots store negated
   originals -v (so ub = min_c W_c is a pure selection; empty scatter
   cells are neutralized by a static +2 bias).

Sharding: constraints are owned by the core that owns their head atom
(atom range of 128 per core), so the head-scatter and clamp are core-local.
The host gathers per-literal value rows between launches (pure layout).
"""
import numpy as np

import concourse.bass as bass
import concourse.tile as tile
from concourse import mybir
from concourse.tile import ScopedClock
from concourse.bass_utils import run_bass_kernel_spmd

B = 128
NCOL = 2048
NA = 1024
C = 512
NCORES = 8
SLOTS = 128          # constraint slots per core (padded)
NLOC = 128           # atoms per core
# value-table regions (row ids):
REG_VP = 0 * NA      # complement-space pos-literal rows (both launches)
REG_VN = 1 * NA      # complement-space neg-literal rows
REG_WP = 2 * NA      # launch-2 neg-head slots: negated pos-literal rows
REG_WN = 3 * NA      # launch-2 neg-head slots: negated neg-literal rows
REG_HP = 4 * NA      # launch-2 pos-head slots: head-activation rows
REG_HN = 5 * NA      # launch-2 neg-head slots: head-activation rows
ZROW = 6 * NA        # neutral (+1) padding row


class FixedTileContext(tile.TileContext):
    """Two workarounds for this walrus/NRT combo: (1) skip the tail
    clear_and_free_semaphores — its InstSemClear makes NRT reject the NEFF at
    load, and NRT resets semaphores per execution anyway; (2) multi-wait
    instructions are split afterwards by split_multi_waits()."""

    def _drain_and_barrier(self, tick_clock, wait_clock):
        drain_inst = self.nc.sync.drain()
        wait_clock.add_sem_waits(
            drain_inst.ins, ScopedClock({None: tick_clock.global_clock})
        )
        self.nc.all_engine_barrier()
        assert self.sems is not None
        popped = self.nc._tile_sem_poison_stack.pop()
        assert popped is self._sem_poison
        self.nc.all_engine_barrier()


def split_multi_waits(nc: bass.Bass) -> int:
    """walrus here accepts only ONE sync wait per instruction; Tile's
    add_semaphores attaches several.  Hoist all but one wait onto fresh
    same-engine nops placed immediately before the instruction (engine
    program order is preserved, so blocking semantics are identical)."""
    n_split = 0
    for f in nc.m.functions:
        for b in f.blocks:
            new = []
            for ins in b.instructions:
                si = ins.sync_info
                waits = list(si.on_wait) if si and si.on_wait else []
                if len(waits) > 1:
                    for w in waits[:-1]:
                        nop = mybir.InstNoOp(
                            name=f"waitsplit-{n_split}", ins=[], outs=[])
                        n_split += 1
                        nop.engine = ins.engine
                        nop.sync_info = mybir.SyncInfo(on_wait=[w], on_update=[])
                        new.append(nop)
                    ins.sync_info = mybir.SyncInfo(
                        on_wait=[waits[-1]],
                        on_update=list(si.on_update) if si.on_update else [])
                new.append(ins)
            b.instructions = new
    return n_split


_PROGRAM_CACHE = {}
SPLIT_WAITS = True  # set False when running under CoreSim / TimelineSim


def strip_preamble(nc: bass.Bass):
    """Remove the const-AP memsets and the initial all-engine barrier from
    the entry block.  Valid because (a) NRT resets semaphores per execution,
    (b) no instruction reads the const APs (activation biases come from our
    own DMA'd blobs)."""
    main = nc.m.functions[0].blocks[0]
    main.instructions = [
        ins for ins in main.instructions
        if not isinstance(ins, (mybir.InstMemset, mybir.InstDrain,
                                mybir.InstEventSemaphore))
    ]


def strip_epilogue(nc: bass.Bass):
    """Keep only the first drain of the end block (it carries the global
    tile-clock sem waits, incl. the output-DMA completion) and drop the two
    all-engine barrier rounds behind it."""
    for blk in nc.m.functions[0].blocks:
        if not blk.name.endswith("_end"):
            continue
        kept, seen_drain = [], False
        for ins in blk.instructions:
            if isinstance(ins, mybir.InstDrain):
                if not seen_drain:
                    kept.append(ins)
                    seen_drain = True
                continue
            if isinstance(ins, mybir.InstEventSemaphore):
                continue
            kept.append(ins)
        blk.instructions = kept


def _col_min_tree(nc, pool, src, k, out_ap, name, dt):
    """Min over the k columns of src ([64, k, B] AP, base partition 0),
    written into out_ap ([64, B], any base partition).  Uses bulk
    first-half-vs-second-half tensor_tensor ops (equal input base
    partitions — required by the BIR verifier)."""
    mn = mybir.AluOpType.min
    cur, i = src, 0
    while k > 3:
        assert k % 2 == 0, f"host must pad col count even, got {k}"
        h = k // 2
        t = pool.tile([64, h, B], dt, tag=f"{name}t{i}")
        nc.vector.tensor_tensor(t[:], cur[:, 0:h, :], cur[:, h:2 * h, :], mn)
        cur, k, i = t[:], h, i + 1
    if k == 3:
        t = pool.tile([64, B], dt, tag=f"{name}p")
        nc.vector.tensor_tensor(t[:], cur[:, 0, :], cur[:, 1, :], mn)
        nc.vector.tensor_tensor(out_ap, t[:], cur[:, 2, :], mn)
    elif k == 2:
        nc.vector.tensor_tensor(out_ap, cur[:, 0, :], cur[:, 1, :], mn)
    else:
        nc.vector.tensor_tensor(out_ap, cur[:, 0, :], cur[:, 0, :], mn)


def _build_p1(KH: int, KL: int, lpos: int, lneg: int) -> bass.Bass:
    """Launch-1 program: f32 complement-space pipeline (u1 and 1-u1 must
    both stay relatively accurate for apply-2's tables)."""
    key = ("p1", KH, KL, lpos, lneg)
    if key in _PROGRAM_CACHE:
        return _PROGRAM_CACHE[key]

    f32, bf16 = mybir.dt.float32, mybir.dt.bfloat16
    mx, mn = mybir.AluOpType.max, mybir.AluOpType.min
    L = lpos + lneg
    nc = bass.Bass(num_devices=NCORES)
    vah_d = nc.declare_dram_parameter("vah", [64, KH * B], f32, isOutput=False)
    val_d = nc.declare_dram_parameter("val", [64, KL * B], f32, isOutput=False)
    vb_d = nc.declare_dram_parameter("vb", [128, L * NLOC], bf16, isOutput=False)
    bs_d = nc.declare_dram_parameter("bs", [NLOC, B], f32, isOutput=False)
    u_d = nc.declare_dram_parameter("u", [NLOC, B], f32, isOutput=True)

    with FixedTileContext(nc) as tc:
        with (
            tc.tile_pool(name="sbuf", bufs=1) as pool,
            tc.tile_pool(name="psum", bufs=1, space="PSUM") as psum,
        ):
            vah = pool.tile([64, KH, B], f32)
            nc.sync.dma_start(vah[:], vah_d[:].rearrange("p (k b) -> p k b", k=KH))
            val = pool.tile([64, KL, B], f32)
            nc.sync.dma_start(val[:], val_d[:].rearrange("p (k b) -> p k b", k=KL))
            vb = pool.tile([128, L, NLOC], bf16)
            nc.sync.dma_start(
                vb[:], vb_d[:].rearrange("p (l n) -> p l n", l=L))
            bs = pool.tile([NLOC, B], f32)
            nc.scalar.dma_start(bs[:], bs_d[:])

            # --- per-slot bm: hi slots -> bm[0:64], lo slots -> bm[64:128]
            bm = pool.tile([128, B], f32)
            _col_min_tree(nc, pool, vah[:], KH, bm[0:64, :], "h", f32)
            _col_min_tree(nc, pool, val[:], KL, bm[64:128, :], "l", f32)

            # split f32 bm into bf16 hi+lo for fast PE transport
            bmh = pool.tile([128, B], bf16)
            nc.vector.tensor_scalar(bmh[:], bm[:], 0.0, None, mybir.AluOpType.add)
            bml = pool.tile([128, B], bf16)
            nc.vector.tensor_tensor(bml[:], bm[:], bmh[:], mybir.AluOpType.subtract)

            ps = []
            for l in range(L):
                pt = psum.tile([NLOC, B], f32, tag=f"ps{l}")
                nc.tensor.matmul(pt[:], vb[:, l, :], bmh[:], start=True, stop=False)
                nc.tensor.matmul(pt[:], vb[:, l, :], bml[:], start=False, stop=True)
                ps.append(pt)

            # relu(ps) on the Activation engine == max(0, contribution)
            rl = []
            for l in range(L):
                t = pool.tile([NLOC, B], f32, tag=f"r{l}")
                nc.scalar.activation(
                    t[:], ps[l][:], mybir.ActivationFunctionType.Relu)
                rl.append(t)
            lb = rl[0]
            for l in range(1, lpos):
                nxt = pool.tile([NLOC, B], f32, tag=f"lb{l}")
                nc.vector.tensor_tensor(nxt[:], lb[:], rl[l][:], mx)
                lb = nxt
            nm = rl[lpos]
            for l in range(1, lneg):
                nxt = pool.tile([NLOC, B], f32, tag=f"nm{l}")
                nc.vector.tensor_tensor(nxt[:], nm[:], rl[lpos + l][:], mx)
                nm = nxt
            ub = pool.tile([NLOC, B], f32)
            nc.vector.tensor_scalar(
                ub[:], nm[:], -1.0, 1.0, mybir.AluOpType.mult, mybir.AluOpType.add)

            lo = pool.tile([NLOC, B], f32)
            nc.vector.tensor_tensor(lo[:], lb[:], ub[:], mn)
            hi = pool.tile([NLOC, B], f32)
            nc.vector.tensor_tensor(hi[:], lb[:], ub[:], mx)
            mid = pool.tile([NLOC, B], f32)
            nc.vector.tensor_tensor(mid[:], hi[:], bs[:], mn)
            u = pool.tile([NLOC, B], f32)
            nc.vector.tensor_tensor(u[:], lo[:], mid[:], mx)
            nc.scalar.dma_start(u_d[:], u[:])

    if SPLIT_WAITS:
        split_multi_waits(nc)
    _PROGRAM_CACHE[key] = nc
    return nc


def _build_p2(KH: int, KL: int, lpos: int, lneg: int) -> bass.Bass:
    """Launch-2 program: bf16, head-sign-specific value spaces."""
    key = ("p2", KH, KL, lpos, lneg)
    if key in _PROGRAM_CACHE:
        return _PROGRAM_CACHE[key]

    f32, bf16 = mybir.dt.float32, mybir.dt.bfloat16
    mx, mn = mybir.AluOpType.max, mybir.AluOpType.min
    L = lpos + lneg
    K = KH + KL
    # scat layers | base | neg-layer biases (f32 bit-packed as bf16 pairs)
    VBW = (L + 1) * NLOC + 2 * lneg
    nc = bass.Bass(num_devices=NCORES)
    vah_d = nc.declare_dram_parameter("vah", [64, KH * B], bf16, isOutput=False)
    val_d = nc.declare_dram_parameter("val", [64, KL * B], bf16, isOutput=False)
    vb_d = nc.declare_dram_parameter("vb", [128, VBW], bf16, isOutput=False)
    u_d = nc.declare_dram_parameter("u", [NLOC, B], bf16, isOutput=True)

    with FixedTileContext(nc) as tc:
        with (
            tc.tile_pool(name="sbuf", bufs=1) as pool,
            tc.tile_pool(name="psum", bufs=1, space="PSUM") as psum,
        ):
            vah = pool.tile([64, KH, B], bf16)
            nc.sync.dma_start(vah[:], vah_d[:].rearrange("p (k b) -> p k b", k=KH))
            val = pool.tile([64, KL, B], bf16)
            nc.sync.dma_start(val[:], val_d[:].rearrange("p (k b) -> p k b", k=KL))
            vb = pool.tile([128, VBW], bf16)
            nc.sync.dma_start(vb[:], vb_d[:])

            bm = pool.tile([128, B], bf16)
            _col_min_tree(nc, pool, vah[:], KH, bm[0:64, :], "h", bf16)
            _col_min_tree(nc, pool, val[:], KL, bm[64:128, :], "l", bf16)

            ps = []
            for l in range(L):
                pt = psum.tile([NLOC, B], f32, tag=f"ps{l}")
                nc.tensor.matmul(pt[:], vb[:, l * NLOC:(l + 1) * NLOC],
                                 bm[:], start=True, stop=True)
                ps.append(pt)

            # lb = max_l relu(ps_l): relus on Act (exactly max(0,.)),
            # maxes on DVE
            rl = []
            for l in range(lpos):
                t = pool.tile([NLOC, B], bf16, tag=f"r{l}")
                nc.scalar.activation(
                    t[:], ps[l][:], mybir.ActivationFunctionType.Relu)
                rl.append(t)
            lb = rl[0]
            for l in range(1, lpos):
                nxt = pool.tile([NLOC, B], bf16, tag=f"lb{l}")
                nc.vector.tensor_tensor(nxt[:], lb[:], rl[l][:], mx)
                lb = nxt

            # ub = min_l (bias_l - ps): ps holds -W per cell; static bias
            # (+2 on empty cells, else 0) keeps empties neutral for the min
            nms = []
            boff = (L + 1) * NLOC
            for l in range(lneg):
                nm = pool.tile([NLOC, B], bf16, tag=f"n{l}")
                bias_ap = vb[:, boff + 2 * l:boff + 2 * l + 2].bitcast(f32)
                nc.vector.tensor_scalar(
                    nm[:], ps[lpos + l][:], -1.0, bias_ap,
                    mybir.AluOpType.mult, mybir.AluOpType.add)
                nms.append(nm)
            ub = nms[0]
            for l in range(1, lneg):
                nxt = pool.tile([NLOC, B], bf16, tag=f"ub{l}")
                nc.vector.tensor_tensor(nxt[:], ub[:], nms[l][:], mn)
                ub = nxt

            # u = max(min(lb,ub), min(max(lb,ub), base))
            base = vb[:, L * NLOC:(L + 1) * NLOC]
            lo = pool.tile([NLOC, B], bf16)
            nc.vector.tensor_tensor(lo[:], lb[:], ub[:], mn)
            hi = pool.tile([NLOC, B], bf16)
            nc.vector.tensor_tensor(hi[:], lb[:], ub[:], mx)
            mid = pool.tile([NLOC, B], bf16)
            nc.vector.tensor_tensor(mid[:], hi[:], base, mn)
            u = pool.tile([NLOC, B], bf16)
            nc.vector.tensor_tensor(u[:], lo[:], mid[:], mx)
            nc.scalar.dma_start(u_d[:], u[:])

    if SPLIT_WAITS:
        split_multi_waits(nc)
    _PROGRAM_CACHE[key] = nc
    return nc


def _pad_k(k):
    """Smallest col count >= k that the bulk halving tree accepts
    (k = m * 2^j with m in {1,2,3})."""
    if k <= 3:
        return k
    c = 4
    while True:
        for m in (4, 6):
            if m * c // 4 >= k:
                return m * c // 4
        c *= 2


def _pack_rows(row_lists_core, KH, KL):
    """hi slots (0:64) -> rows_hi[s] cols 0..KH; lo slots (64:128) ->
    rows_lo[s-64] cols 0..KL; ZROW pads."""
    rows_hi = np.full((64, KH), ZROW, dtype=np.int64)
    rows_lo = np.full((64, KL), ZROW, dtype=np.int64)
    for s, rr in enumerate(row_lists_core):
        if s < 64:
            rows_hi[s, :len(rr)] = rr
        else:
            rows_lo[s - 64, :len(rr)] = rr
    return rows_hi, rows_lo


class _Prep:
    """Host-side, launch-independent preprocessing (slot assignment, literal
    row ids, scatter one-hots)."""

    def __init__(self, preds, goal, atoms, pos_body, neg_body, pos_head, neg_head):
        f32 = np.float32
        import ml_dtypes
        self.bf16 = ml_dtypes.bfloat16
        self.atoms = np.asarray(atoms)
        self.p = preds[:, self.atoms].astype(f32)            # [B, NA]
        self.g = goal[:, self.atoms].astype(f32)
        self.pT = np.ascontiguousarray(self.p.T)             # [NA, B]
        self.gT = np.ascontiguousarray(self.g.T)

        hsum = pos_head + neg_head
        assert np.all(hsum.sum(axis=1) == 1.0), "heads must be one-hot"
        self.h = np.argmax(hsum, axis=1)                     # [C]
        self.head_is_pos = pos_head[np.arange(C), self.h] == 1.0
        owner = self.h // NLOC

        pos_lists = [np.nonzero(pos_body[c])[0] for c in range(C)]
        neg_lists = [np.nonzero(neg_body[c])[0] for c in range(C)]
        # launch-1 rows: complement space for every slot, no head rows
        row1 = [np.concatenate([REG_VP + pos_lists[c], REG_VN + neg_lists[c]])
                .astype(np.int64) for c in range(C)]
        # launch-2 rows: sign-specific space + head-activation row
        row2 = []
        for c in range(C):
            if self.head_is_pos[c]:
                rp, rn, rh = REG_VP, REG_VN, REG_HP
            else:
                rp, rn, rh = REG_WP, REG_WN, REG_HN
            row2.append(np.concatenate([
                rp + pos_lists[c], rn + neg_lists[c],
                [rh + self.h[c]]]).astype(np.int64))
        ncnt = np.array([len(r) for r in row2])

        self.cons = []
        k1h = k1l = k2h = k2l = 1
        for i in range(NCORES):
            ci = np.nonzero(owner == i)[0]
            assert len(ci) <= SLOTS, f"core {i} has {len(ci)} constraints"
            ci = ci[np.argsort(-ncnt[ci], kind="stable")]
            self.cons.append(ci)
            c2 = ncnt[ci]
            k2h = max(k2h, int(c2[:64].max(initial=0)))
            k2l = max(k2l, int(c2[64:].max(initial=0)))
            c1 = c2 - 1                                      # no head row
            k1h = max(k1h, int(c1[:64].max(initial=0)))
            k1l = max(k1l, int(c1[64:].max(initial=0)))
        self.key1 = (_pad_k(k1h), _pad_k(k1l))
        self.key2 = (_pad_k(k2h), _pad_k(k2l))

        self.rows1 = []
        self.rows2 = []
        lpos_need = lneg_need = 1
        layer_asn = []
        for i in range(NCORES):
            ci = self.cons[i]
            self.rows1.append(_pack_rows([row1[c] for c in ci], *self.key1))
            self.rows2.append(_pack_rows([row2[c] for c in ci], *self.key2))

            counts = {}
            asn = []
            for s, c in enumerate(ci):
                key = (self.h[c] % NLOC, bool(self.head_is_pos[c]))
                l = counts.get(key, 0)
                counts[key] = l + 1
                asn.append((s, key[1], l, key[0]))
                if key[1]:
                    lpos_need = max(lpos_need, l + 1)
                else:
                    lneg_need = max(lneg_need, l + 1)
            layer_asn.append(asn)

        self.lpos, self.lneg = lpos_need, lneg_need
        L = self.lpos + self.lneg
        self.scat = []        # per core: [128, L*NLOC] bf16 one-hot layers
        self.negbias = []     # per core: [NLOC, lneg] bf16 (+2 on empty cell)
        for i in range(NCORES):
            sc = np.zeros((SLOTS, L, NLOC), dtype=f32)
            occ = np.zeros((self.lneg, NLOC), dtype=bool)
            for s, is_pos, l, n in layer_asn[i]:
                li = l if is_pos else self.lpos + l
                sc[s, li, n] = 1.0
                if not is_pos:
                    occ[l, n] = True
            self.scat.append(np.ascontiguousarray(
                sc.reshape(SLOTS, L * NLOC)).astype(self.bf16))
            self.negbias.append(np.ascontiguousarray(
                np.where(occ.T, 0.0, 2.0).astype(f32)))

    def gather(self, vcat, rows, KH, KL, dtype):
        out = []
        for i in range(NCORES):
            rh, rl = rows[i]
            out.append((
                np.ascontiguousarray(
                    vcat[rh].reshape(64, KH * B)).astype(dtype),
                np.ascontiguousarray(
                    vcat[rl].reshape(64, KL * B)).astype(dtype),
            ))
        return out


def kernel(preds, goal, atoms, pos_body, neg_body, pos_head, neg_head):
    preds = np.asarray(preds)
    f32 = np.float32
    prep = _Prep(np.asarray(preds, f32), np.asarray(goal, f32),
                 atoms, np.asarray(pos_body, f32),
                 np.asarray(neg_body, f32),
                 np.asarray(pos_head, f32),
                 np.asarray(neg_head, f32))
    nc1 = _build_p1(*prep.key1, prep.lpos, prep.lneg)
    nc2 = _build_p2(*prep.key2, prep.lpos, prep.lneg)
    core_ids = list(range(NCORES))
    g, p = prep.gT, prep.pT

    # ---- launch 1 (f32): complement-space table, goal-folded actives ----
    vcat1 = np.concatenate([
        np.where(g == 1.0, p, -1.0),             # v'+ = 1 - (1-p), sat-gated
        np.where(g == 0.0, 1.0 - p, -1.0),       # v'- = 1 - p-lit
        np.ones((4 * NA + 1, B), f32),           # unused regions + pad row
    ], axis=0)
    vas = prep.gather(vcat1, prep.rows1, *prep.key1, f32)
    in_maps = [{"vah": vas[i][0], "val": vas[i][1], "vb": prep.scat[i],
                "bs": np.ascontiguousarray(p[i * NLOC:(i + 1) * NLOC])}
               for i in range(NCORES)]
    res = run_bass_kernel_spmd(nc1, in_maps, core_ids)
    u1T = np.concatenate(
        [res.results[i]["u"].astype(f32) for i in range(NCORES)], axis=0)

    # ---- launch 2 (bf16): sign-specific spaces + head-activation rows ----
    a_ = (1.0 - g) * (1.0 - u1T)
    b_ = g * u1T
    vcat2 = np.concatenate([
        1.0 - a_,                                # v'+   (pos-head slots)
        1.0 - b_,                                # v'-
        -a_,                                     # -v+   (neg-head slots)
        -b_,                                     # -v-
        1.0 - 2.0 * g,                           # head-act, pos slots
        2.0 * g - 2.0,                           # head-act, neg slots
        np.ones((1, B), f32),                    # pad row
    ], axis=0)
    vas = prep.gather(vcat2, prep.rows2, *prep.key2, prep.bf16)
    in_maps = []
    for i in range(NCORES):
        vb = np.concatenate([
            prep.scat[i],
            np.ascontiguousarray(
                u1T[i * NLOC:(i + 1) * NLOC]).astype(prep.bf16),
            prep.negbias[i].view(prep.bf16),   # f32 bits as bf16 pairs
        ], axis=1)
        in_maps.append({"vah": vas[i][0], "val": vas[i][1], "vb": vb})
    res = run_bass_kernel_spmd(nc2, in_maps, core_ids)
    u2T = np.concatenate(
        [res.results[i]["u"].astype(f32) for i in range(NCORES)], axis=0)

    out = np.array(preds, dtype=preds.dtype, copy=True)
    out[:, prep.atoms] = u2T.T.astype(preds.dtype)
    return out


# revision 38
# speedup vs baseline: 1.5909x; 1.1086x over previous
"""Trainium2 Bass kernel for nn_ConstraintsModule (fuzzy-logic constraint
propagation).

Algorithm notes
---------------
The reference computes, twice (apply-1 with active=full_body, apply-2 with
active=unsat_head and goal-masked bodies):

    body_rev[b,c,a] = pb[c,a] + v[b,a]*(nb-pb)      -> max over a
    body_min[b,c]   = active[b,c] * (1 - max_a body_rev)
    lb[n] = max_c body_min * pos_head[c,n] ; ub = 1 - max_c body_min*neg_head
    u = max(min(lb,ub), min(max(lb,ub), v))

Bodies are sparse (~4 literals/constraint) and heads one-hot, so per
constraint we only gather its literal-value rows and min-reduce their
complements:  bm = 1 - max_a(v) = min_a(1 - v).

Key tricks:
1. The `active` gate folds into the gathered VALUES: a literal row whose
   goal-condition fails gets complement value -1, making bm <= 0, and
   relu() at the scatter stage reproduces active=0 exactly.  For apply-2
   (active = head literal unsatisfied by goal) one extra "head
   activation" row is appended per constraint.  This removes the
   goal@body equality matmul and its big operand loads completely.
2. Precision: apply-2 consumes BOTH u1 and 1-u1, so launch 1 runs f32
   end-to-end (table rows are exact f32 complements; the f32 bm crosses
   the PE as a split bf16 pair hi+lo accumulated in PSUM, keeping rel
   error ~1.6e-5).  Launch 2 only needs u2 itself to be accurate, so it
   runs bf16 with head-sign-specific value spaces: pos-head slots store
   complements v'=1-v (bm selections), neg-head slots store negated
   originals -v (so ub = min_c W_c is a pure selection; empty scatter
   cells are neutralized by a static +2 bias).

Sharding: constraints are owned by the core that owns their head atom
(atom range of 128 per core), so the head-scatter and clamp are core-local.
The host gathers per-literal value rows between launches (pure layout).
"""
import numpy as np

import concourse.bass as bass
import concourse.tile as tile
from concourse import mybir
from concourse.tile import ScopedClock
from concourse.bass_utils import run_bass_kernel_spmd

B = 128
NCOL = 2048
NA = 1024
C = 512
NCORES = 8
SLOTS = 128          # constraint slots per core (padded)
NLOC = 128           # atoms per core
# value-table regions (row ids):
REG_VP = 0 * NA      # complement-space pos-literal rows (both launches)
REG_VN = 1 * NA      # complement-space neg-literal rows
REG_WP = 2 * NA      # launch-2 neg-head slots: negated pos-literal rows
REG_WN = 3 * NA      # launch-2 neg-head slots: negated neg-literal rows
REG_HP = 4 * NA      # launch-2 pos-head slots: head-activation rows
REG_HN = 5 * NA      # launch-2 neg-head slots: head-activation rows
ZROW = 6 * NA        # neutral (+1) padding row


class FixedTileContext(tile.TileContext):
    """Two workarounds for this walrus/NRT combo: (1) skip the tail
    clear_and_free_semaphores — its InstSemClear makes NRT reject the NEFF at
    load, and NRT resets semaphores per execution anyway; (2) multi-wait
    instructions are split afterwards by split_multi_waits()."""

    def _drain_and_barrier(self, tick_clock, wait_clock):
        drain_inst = self.nc.sync.drain()
        wait_clock.add_sem_waits(
            drain_inst.ins, ScopedClock({None: tick_clock.global_clock})
        )
        self.nc.all_engine_barrier()
        assert self.sems is not None
        popped = self.nc._tile_sem_poison_stack.pop()
        assert popped is self._sem_poison
        self.nc.all_engine_barrier()


def split_multi_waits(nc: bass.Bass) -> int:
    """walrus here accepts only ONE sync wait per instruction; Tile's
    add_semaphores attaches several.  Hoist all but one wait onto fresh
    same-engine nops placed immediately before the instruction (engine
    program order is preserved, so blocking semantics are identical)."""
    n_split = 0
    for f in nc.m.functions:
        for b in f.blocks:
            new = []
            for ins in b.instructions:
                si = ins.sync_info
                waits = list(si.on_wait) if si and si.on_wait else []
                if len(waits) > 1:
                    for w in waits[:-1]:
                        nop = mybir.InstNoOp(
                            name=f"waitsplit-{n_split}", ins=[], outs=[])
                        n_split += 1
                        nop.engine = ins.engine
                        nop.sync_info = mybir.SyncInfo(on_wait=[w], on_update=[])
                        new.append(nop)
                    ins.sync_info = mybir.SyncInfo(
                        on_wait=[waits[-1]],
                        on_update=list(si.on_update) if si.on_update else [])
                new.append(ins)
            b.instructions = new
    return n_split


_PROGRAM_CACHE = {}
SPLIT_WAITS = True  # set False when running under CoreSim / TimelineSim


def strip_preamble(nc: bass.Bass):
    """Remove the const-AP memsets and the initial all-engine barrier from
    the entry block.  Valid because (a) NRT resets semaphores per execution,
    (b) no instruction reads the const APs (activation biases come from our
    own DMA'd blobs)."""
    main = nc.m.functions[0].blocks[0]
    main.instructions = [
        ins for ins in main.instructions
        if not isinstance(ins, (mybir.InstMemset, mybir.InstDrain,
                                mybir.InstEventSemaphore))
    ]


def strip_epilogue(nc: bass.Bass):
    """Keep only the first drain of the end block (it carries the global
    tile-clock sem waits, incl. the output-DMA completion) and drop the two
    all-engine barrier rounds behind it."""
    for blk in nc.m.functions[0].blocks:
        if not blk.name.endswith("_end"):
            continue
        kept, seen_drain = [], False
        for ins in blk.instructions:
            if isinstance(ins, mybir.InstDrain):
                if not seen_drain:
                    kept.append(ins)
                    seen_drain = True
                continue
            if isinstance(ins, mybir.InstEventSemaphore):
                continue
            kept.append(ins)
        blk.instructions = kept


def _col_min_tree(nc, pool, src, k, out_ap, name, dt):
    """Min over the k columns of src ([64, k, B] AP, base partition 0),
    written into out_ap ([64, B], any base partition).  Uses bulk
    first-half-vs-second-half tensor_tensor ops (equal input base
    partitions — required by the BIR verifier)."""
    mn = mybir.AluOpType.min
    cur, i = src, 0
    while k > 3:
        assert k % 2 == 0, f"host must pad col count even, got {k}"
        h = k // 2
        t = pool.tile([64, h, B], dt, tag=f"{name}t{i}")
        nc.vector.tensor_tensor(t[:], cur[:, 0:h, :], cur[:, h:2 * h, :], mn)
        cur, k, i = t[:], h, i + 1
    if k == 3:
        t = pool.tile([64, B], dt, tag=f"{name}p")
        nc.vector.tensor_tensor(t[:], cur[:, 0, :], cur[:, 1, :], mn)
        nc.vector.tensor_tensor(out_ap, t[:], cur[:, 2, :], mn)
    elif k == 2:
        nc.vector.tensor_tensor(out_ap, cur[:, 0, :], cur[:, 1, :], mn)
    else:
        nc.vector.tensor_tensor(out_ap, cur[:, 0, :], cur[:, 0, :], mn)


def _build_p1(KH: int, KL: int, lpos: int, lneg: int) -> bass.Bass:
    """Launch-1 program: f32 complement-space pipeline (u1 and 1-u1 must
    both stay relatively accurate for apply-2's tables)."""
    key = ("p1", KH, KL, lpos, lneg)
    if key in _PROGRAM_CACHE:
        return _PROGRAM_CACHE[key]

    f32, bf16 = mybir.dt.float32, mybir.dt.bfloat16
    mx, mn = mybir.AluOpType.max, mybir.AluOpType.min
    L = lpos + lneg
    VBW = L * NLOC + 1                   # scat layers | zero bias col
    nc = bass.Bass(num_devices=NCORES)
    vah_d = nc.declare_dram_parameter("vah", [64, KH * B], f32, isOutput=False)
    val_d = nc.declare_dram_parameter("val", [64, KL * B], f32, isOutput=False)
    vb_d = nc.declare_dram_parameter("vb", [128, VBW], bf16, isOutput=False)
    bs_d = nc.declare_dram_parameter("bs", [NLOC, B], f32, isOutput=False)
    u_d = nc.declare_dram_parameter("u", [NLOC, B], f32, isOutput=True)

    with FixedTileContext(nc) as tc:
        with (
            tc.tile_pool(name="sbuf", bufs=1) as pool,
            tc.tile_pool(name="psum", bufs=1, space="PSUM") as psum,
        ):
            vah = pool.tile([64, KH, B], f32)
            nc.sync.dma_start(vah[:], vah_d[:].rearrange("p (k b) -> p k b", k=KH))
            val = pool.tile([64, KL, B], f32)
            nc.sync.dma_start(val[:], val_d[:].rearrange("p (k b) -> p k b", k=KL))
            vb = pool.tile([128, VBW], bf16)
            nc.sync.dma_start(vb[:], vb_d[:])
            bs = pool.tile([NLOC, B], f32)
            nc.scalar.dma_start(bs[:], bs_d[:])
            zbias = vb[:, L * NLOC:L * NLOC + 1]

            # --- per-slot bm: hi slots -> bm[0:64], lo slots -> bm[64:128]
            bm = pool.tile([128, B], f32)
            _col_min_tree(nc, pool, vah[:], KH, bm[0:64, :], "h", f32)
            _col_min_tree(nc, pool, val[:], KL, bm[64:128, :], "l", f32)

            # split f32 bm into bf16 hi+lo for fast PE transport
            bmh = pool.tile([128, B], bf16)
            nc.vector.tensor_scalar(bmh[:], bm[:], 0.0, None, mybir.AluOpType.add)
            bml = pool.tile([128, B], bf16)
            nc.vector.tensor_tensor(bml[:], bm[:], bmh[:], mybir.AluOpType.subtract)

            ps = []
            for l in range(L):
                pt = psum.tile([NLOC, B], f32, tag=f"ps{l}")
                sc_l = vb[:, l * NLOC:(l + 1) * NLOC]
                nc.tensor.matmul(pt[:], sc_l, bmh[:], start=True, stop=False)
                nc.tensor.matmul(pt[:], sc_l, bml[:], start=False, stop=True)
                ps.append(pt)

            # relu(ps) on the Activation engine == max(0, contribution)
            rl = []
            for l in range(L):
                t = pool.tile([NLOC, B], f32, tag=f"r{l}")
                nc.scalar.activation(
                    t[:], ps[l][:], mybir.ActivationFunctionType.Relu,
                    bias=zbias)
                rl.append(t)
            lb = rl[0]
            for l in range(1, lpos):
                nxt = pool.tile([NLOC, B], f32, tag=f"lb{l}")
                nc.vector.tensor_tensor(nxt[:], lb[:], rl[l][:], mx)
                lb = nxt
            nm = rl[lpos]
            for l in range(1, lneg):
                nxt = pool.tile([NLOC, B], f32, tag=f"nm{l}")
                nc.vector.tensor_tensor(nxt[:], nm[:], rl[lpos + l][:], mx)
                nm = nxt
            ub = pool.tile([NLOC, B], f32)
            nc.vector.tensor_scalar(
                ub[:], nm[:], -1.0, 1.0, mybir.AluOpType.mult, mybir.AluOpType.add)

            lo = pool.tile([NLOC, B], f32)
            nc.vector.tensor_tensor(lo[:], lb[:], ub[:], mn)
            hi = pool.tile([NLOC, B], f32)
            nc.vector.tensor_tensor(hi[:], lb[:], ub[:], mx)
            mid = pool.tile([NLOC, B], f32)
            nc.vector.tensor_tensor(mid[:], hi[:], bs[:], mn)
            u = pool.tile([NLOC, B], f32)
            nc.vector.tensor_tensor(u[:], lo[:], mid[:], mx)
            nc.scalar.dma_start(u_d[:], u[:])

    strip_preamble(nc)
    strip_epilogue(nc)
    if SPLIT_WAITS:
        split_multi_waits(nc)
    _PROGRAM_CACHE[key] = nc
    return nc


def _build_p2(KH: int, KL: int, lpos: int, lneg: int) -> bass.Bass:
    """Launch-2 program: bf16, head-sign-specific value spaces."""
    key = ("p2", KH, KL, lpos, lneg)
    if key in _PROGRAM_CACHE:
        return _PROGRAM_CACHE[key]

    f32, bf16 = mybir.dt.float32, mybir.dt.bfloat16
    mx, mn = mybir.AluOpType.max, mybir.AluOpType.min
    L = lpos + lneg
    K = KH + KL
    # scat layers | base | neg-layer biases (f32 bits as bf16 pairs) |
    # zero bias col | pad to even width (f32 bitcast needs even stride)
    VBW = (L + 1) * NLOC + 2 * lneg + 2
    nc = bass.Bass(num_devices=NCORES)
    vah_d = nc.declare_dram_parameter("vah", [64, KH * B], bf16, isOutput=False)
    val_d = nc.declare_dram_parameter("val", [64, KL * B], bf16, isOutput=False)
    vb_d = nc.declare_dram_parameter("vb", [128, VBW], bf16, isOutput=False)
    u_d = nc.declare_dram_parameter("u", [NLOC, B], bf16, isOutput=True)

    with FixedTileContext(nc) as tc:
        with (
            tc.tile_pool(name="sbuf", bufs=1) as pool,
            tc.tile_pool(name="psum", bufs=1, space="PSUM") as psum,
        ):
            vah = pool.tile([64, KH, B], bf16)
            nc.sync.dma_start(vah[:], vah_d[:].rearrange("p (k b) -> p k b", k=KH))
            val = pool.tile([64, KL, B], bf16)
            nc.sync.dma_start(val[:], val_d[:].rearrange("p (k b) -> p k b", k=KL))
            vb = pool.tile([128, VBW], bf16)
            nc.sync.dma_start(vb[:], vb_d[:])

            bm = pool.tile([128, B], bf16)
            _col_min_tree(nc, pool, vah[:], KH, bm[0:64, :], "h", bf16)
            _col_min_tree(nc, pool, val[:], KL, bm[64:128, :], "l", bf16)

            ps = []
            for l in range(L):
                pt = psum.tile([NLOC, B], f32, tag=f"ps{l}")
                nc.tensor.matmul(pt[:], vb[:, l * NLOC:(l + 1) * NLOC],
                                 bm[:], start=True, stop=True)
                ps.append(pt)

            # lb = max_l relu(ps_l): relus on Act (exactly max(0,.)),
            # maxes on DVE
            zbias = vb[:, VBW - 2:VBW - 1]
            rl = []
            for l in range(lpos):
                t = pool.tile([NLOC, B], bf16, tag=f"r{l}")
                nc.scalar.activation(
                    t[:], ps[l][:], mybir.ActivationFunctionType.Relu,
                    bias=zbias)
                rl.append(t)
            lb = rl[0]
            for l in range(1, lpos):
                nxt = pool.tile([NLOC, B], bf16, tag=f"lb{l}")
                nc.vector.tensor_tensor(nxt[:], lb[:], rl[l][:], mx)
                lb = nxt

            # ub = min_l (bias_l - ps): ps holds -W per cell; static bias
            # (+2 on empty cells, else 0) keeps empties neutral for the min
            nms = []
            boff = (L + 1) * NLOC
            for l in range(lneg):
                nm = pool.tile([NLOC, B], bf16, tag=f"n{l}")
                bias_ap = vb[:, boff + 2 * l:boff + 2 * l + 2].bitcast(f32)
                nc.vector.tensor_scalar(
                    nm[:], ps[lpos + l][:], -1.0, bias_ap,
                    mybir.AluOpType.mult, mybir.AluOpType.add)
                nms.append(nm)
            ub = nms[0]
            for l in range(1, lneg):
                nxt = pool.tile([NLOC, B], bf16, tag=f"ub{l}")
                nc.vector.tensor_tensor(nxt[:], ub[:], nms[l][:], mn)
                ub = nxt

            # u = max(min(lb,ub), min(max(lb,ub), base))
            base = vb[:, L * NLOC:(L + 1) * NLOC]
            lo = pool.tile([NLOC, B], bf16)
            nc.vector.tensor_tensor(lo[:], lb[:], ub[:], mn)
            hi = pool.tile([NLOC, B], bf16)
            nc.vector.tensor_tensor(hi[:], lb[:], ub[:], mx)
            mid = pool.tile([NLOC, B], bf16)
            nc.vector.tensor_tensor(mid[:], hi[:], base, mn)
            u = pool.tile([NLOC, B], bf16)
            nc.vector.tensor_tensor(u[:], lo[:], mid[:], mx)
            nc.scalar.dma_start(u_d[:], u[:])

    strip_preamble(nc)
    strip_epilogue(nc)
    if SPLIT_WAITS:
        split_multi_waits(nc)
    _PROGRAM_CACHE[key] = nc
    return nc


def _pad_k(k):
    """Smallest col count >= k that the bulk halving tree accepts
    (k = m * 2^j with m in {1,2,3})."""
    if k <= 3:
        return k
    c = 4
    while True:
        for m in (4, 6):
            if m * c // 4 >= k:
                return m * c // 4
        c *= 2


def _pack_rows(row_lists_core, KH, KL):
    """hi slots (0:64) -> rows_hi[s] cols 0..KH; lo slots (64:128) ->
    rows_lo[s-64] cols 0..KL; ZROW pads."""
    rows_hi = np.full((64, KH), ZROW, dtype=np.int64)
    rows_lo = np.full((64, KL), ZROW, dtype=np.int64)
    for s, rr in enumerate(row_lists_core):
        if s < 64:
            rows_hi[s, :len(rr)] = rr
        else:
            rows_lo[s - 64, :len(rr)] = rr
    return rows_hi, rows_lo


class _Prep:
    """Host-side, launch-independent preprocessing (slot assignment, literal
    row ids, scatter one-hots)."""

    def __init__(self, preds, goal, atoms, pos_body, neg_body, pos_head, neg_head):
        f32 = np.float32
        import ml_dtypes
        self.bf16 = ml_dtypes.bfloat16
        self.atoms = np.asarray(atoms)
        self.p = preds[:, self.atoms].astype(f32)            # [B, NA]
        self.g = goal[:, self.atoms].astype(f32)
        self.pT = np.ascontiguousarray(self.p.T)             # [NA, B]
        self.gT = np.ascontiguousarray(self.g.T)

        hsum = pos_head + neg_head
        assert np.all(hsum.sum(axis=1) == 1.0), "heads must be one-hot"
        self.h = np.argmax(hsum, axis=1)                     # [C]
        self.head_is_pos = pos_head[np.arange(C), self.h] == 1.0
        owner = self.h // NLOC

        pos_lists = [np.nonzero(pos_body[c])[0] for c in range(C)]
        neg_lists = [np.nonzero(neg_body[c])[0] for c in range(C)]
        # launch-1 rows: complement space for every slot, no head rows
        row1 = [np.concatenate([REG_VP + pos_lists[c], REG_VN + neg_lists[c]])
                .astype(np.int64) for c in range(C)]
        # launch-2 rows: sign-specific space + head-activation row
        row2 = []
        for c in range(C):
            if self.head_is_pos[c]:
                rp, rn, rh = REG_VP, REG_VN, REG_HP
            else:
                rp, rn, rh = REG_WP, REG_WN, REG_HN
            row2.append(np.concatenate([
                rp + pos_lists[c], rn + neg_lists[c],
                [rh + self.h[c]]]).astype(np.int64))
        ncnt = np.array([len(r) for r in row2])

        self.cons = []
        k1h = k1l = k2h = k2l = 1
        for i in range(NCORES):
            ci = np.nonzero(owner == i)[0]
            assert len(ci) <= SLOTS, f"core {i} has {len(ci)} constraints"
            ci = ci[np.argsort(-ncnt[ci], kind="stable")]
            self.cons.append(ci)
            c2 = ncnt[ci]
            k2h = max(k2h, int(c2[:64].max(initial=0)))
            k2l = max(k2l, int(c2[64:].max(initial=0)))
            c1 = c2 - 1                                      # no head row
            k1h = max(k1h, int(c1[:64].max(initial=0)))
            k1l = max(k1l, int(c1[64:].max(initial=0)))
        self.key1 = (_pad_k(k1h), _pad_k(k1l))
        self.key2 = (_pad_k(k2h), _pad_k(k2l))

        self.rows1 = []
        self.rows2 = []
        lpos_need = lneg_need = 1
        layer_asn = []
        for i in range(NCORES):
            ci = self.cons[i]
            self.rows1.append(_pack_rows([row1[c] for c in ci], *self.key1))
            self.rows2.append(_pack_rows([row2[c] for c in ci], *self.key2))

            counts = {}
            asn = []
            for s, c in enumerate(ci):
                key = (self.h[c] % NLOC, bool(self.head_is_pos[c]))
                l = counts.get(key, 0)
                counts[key] = l + 1
                asn.append((s, key[1], l, key[0]))
                if key[1]:
                    lpos_need = max(lpos_need, l + 1)
                else:
                    lneg_need = max(lneg_need, l + 1)
            layer_asn.append(asn)

        self.lpos, self.lneg = lpos_need, lneg_need
        L = self.lpos + self.lneg
        self.scat = []        # per core: [128, L*NLOC] bf16 one-hot layers
        self.negbias = []     # per core: [NLOC, lneg] bf16 (+2 on empty cell)
        for i in range(NCORES):
            sc = np.zeros((SLOTS, L, NLOC), dtype=f32)
            occ = np.zeros((self.lneg, NLOC), dtype=bool)
            for s, is_pos, l, n in layer_asn[i]:
                li = l if is_pos else self.lpos + l
                sc[s, li, n] = 1.0
                if not is_pos:
                    occ[l, n] = True
            self.scat.append(np.ascontiguousarray(
                sc.reshape(SLOTS, L * NLOC)).astype(self.bf16))
            self.negbias.append(np.ascontiguousarray(
                np.where(occ.T, 0.0, 2.0).astype(f32)))

    def gather(self, vcat, rows, KH, KL, dtype):
        out = []
        for i in range(NCORES):
            rh, rl = rows[i]
            out.append((
                np.ascontiguousarray(
                    vcat[rh].reshape(64, KH * B)).astype(dtype),
                np.ascontiguousarray(
                    vcat[rl].reshape(64, KL * B)).astype(dtype),
            ))
        return out


def kernel(preds, goal, atoms, pos_body, neg_body, pos_head, neg_head):
    preds = np.asarray(preds)
    f32 = np.float32
    prep = _Prep(np.asarray(preds, f32), np.asarray(goal, f32),
                 atoms, np.asarray(pos_body, f32),
                 np.asarray(neg_body, f32),
                 np.asarray(pos_head, f32),
                 np.asarray(neg_head, f32))
    nc1 = _build_p1(*prep.key1, prep.lpos, prep.lneg)
    nc2 = _build_p2(*prep.key2, prep.lpos, prep.lneg)
    core_ids = list(range(NCORES))
    g, p = prep.gT, prep.pT

    # ---- launch 1 (f32): complement-space table, goal-folded actives ----
    vcat1 = np.concatenate([
        np.where(g == 1.0, p, -1.0),             # v'+ = 1 - (1-p), sat-gated
        np.where(g == 0.0, 1.0 - p, -1.0),       # v'- = 1 - p-lit
        np.ones((4 * NA + 1, B), f32),           # unused regions + pad row
    ], axis=0)
    vas = prep.gather(vcat1, prep.rows1, *prep.key1, f32)
    zcol = np.zeros((128, 1), prep.bf16)
    in_maps = [{"vah": vas[i][0], "val": vas[i][1],
                "vb": np.concatenate([prep.scat[i], zcol], axis=1),
                "bs": np.ascontiguousarray(p[i * NLOC:(i + 1) * NLOC])}
               for i in range(NCORES)]
    res = run_bass_kernel_spmd(nc1, in_maps, core_ids)
    u1T = np.concatenate(
        [res.results[i]["u"].astype(f32) for i in range(NCORES)], axis=0)

    # ---- launch 2 (bf16): sign-specific spaces + head-activation rows ----
    a_ = (1.0 - g) * (1.0 - u1T)
    b_ = g * u1T
    vcat2 = np.concatenate([
        1.0 - a_,                                # v'+   (pos-head slots)
        1.0 - b_,                                # v'-
        -a_,                                     # -v+   (neg-head slots)
        -b_,                                     # -v-
        1.0 - 2.0 * g,                           # head-act, pos slots
        2.0 * g - 2.0,                           # head-act, neg slots
        np.ones((1, B), f32),                    # pad row
    ], axis=0)
    vas = prep.gather(vcat2, prep.rows2, *prep.key2, prep.bf16)
    in_maps = []
    for i in range(NCORES):
        vb = np.concatenate([
            prep.scat[i],
            np.ascontiguousarray(
                u1T[i * NLOC:(i + 1) * NLOC]).astype(prep.bf16),
            prep.negbias[i].view(prep.bf16),   # f32 bits as bf16 pairs
            zcol, zcol,
        ], axis=1)
        in_maps.append({"vah": vas[i][0], "val": vas[i][1], "vb": vb})
    res = run_bass_kernel_spmd(nc2, in_maps, core_ids)
    u2T = np.concatenate(
        [res.results[i]["u"].astype(f32) for i in range(NCORES)], axis=0)

    out = np.array(preds, dtype=preds.dtype, copy=True)
    out[:, prep.atoms] = u2T.T.astype(preds.dtype)
    return out
